# revision 15
# baseline (speedup 1.0000x reference)
"""Trainium2 Bass kernel for EpsilonNetGM score function (8-core data parallel).

Closed form of the score (no autodiff):
  acp = alphas_cumprod[t]; mu_k = sqrt(acp)*means_k
  Sigma_k = (1-acp) I + acp covs_k ; L = chol(Sigma); Linv = L^-1; P = Linv^T Linv
  z_k(x) = Linv_k x
  l_k(x) = -0.5|z_k|^2 + (P_k mu_k).x + c'_k        (c' folds logdet, weights, mu)
  r = softmax_k(l)     (computed as exp(l - logsumexp l), no per-column max --
                        a global shift keeps exp in fp32 range)
  out = sqrt(1-acp) * [ sum_k Linv_k^T (r_k z_k) - sum_k r_k (P_k mu_k) ]

v2 layout: x arrives HOST-TRANSPOSED as xT [64, BP] so no on-device transpose.
Partition p = 8k+ds (ds in [0,8)), d = 8t+ds over NT=8 subtiles; 512-col chunks.
Per chunk:
  PE : Z waves (fp32r), maha ones-block reduce + h.x into lT [16,512] psum,
       s-sum matmul, ERep replicate of lnorm, Z recompute, negHsRep + mm2 (bf16)
  ACT: squares (PSUM->SBUF), exp(lT + c'), ln(s), exp(lnorm rep) -> r replicated
  DVE: lnorm = lT - ln(s), W = Z*r (bf16 out), final psum evac
"""

import math
import sys

import numpy as np

sys.path.insert(0, "/opt/trn_rl_repo")

import concourse.bass as bass  # noqa: E402
import concourse.tile as tile  # noqa: E402
from concourse import mybir  # noqa: E402
from concourse.bass_utils import run_bass_kernel_spmd  # noqa: E402

B, K, D, T = 65536, 16, 64, 1000
NCORES = 8
BP = B // NCORES          # rows per core = 8192
NB = 512                  # batch chunk (free dim)
NCHUNK = BP // NB         # 16
DS = 8                    # d-subtile width; partition p = 8*k + ds
NT = D // DS              # 8 subtiles
SHIFT = 40.0              # global exp shift; keeps exp(l) in fp32 range

ROWALT = True             # alternate Z stationaries between row halves
COLTILE = False           # col-tiled mm2 with both-psum final add

F32 = mybir.dt.float32
F32R = mybir.dt.float32r
BF16 = mybir.dt.bfloat16

# f32 cblob column layout
_A1, _ONB, _H2C, _EREP, _ONR, _NHR, _CB, _ID = (
    0, 1024, 1040, 1056, 1184, 1312, 1376, 1377)
CBLOB_W = _ID + 32        # 1409


def _host_precompute(means, weights, covs, alphas_cumprod, t):
    acp = float(np.asarray(alphas_cumprod)[int(t)])
    s1 = math.sqrt(acp)
    sqrt1m = math.sqrt(1.0 - acp)
    mu = (s1 * means).astype(np.float64)
    covs = covs.astype(np.float64)
    sigma = (1.0 - acp) * np.eye(D) + acp * covs
    chol = np.linalg.cholesky(sigma)
    Linv = np.stack([np.linalg.solve(chol[k], np.eye(D)) for k in range(K)])
    P = np.einsum("kdi,kdj->kij", Linv, Linv)
    h = np.einsum("kij,kj->ki", P, mu)
    logdet = 2.0 * np.log(np.diagonal(chol, axis1=1, axis2=2)).sum(-1)
    w = weights.astype(np.float64)
    logw = np.log(w) - math.log(w.sum())
    c = logw - 0.5 * (D * math.log(2 * math.pi) + logdet)
    cp = c - 0.5 * np.einsum("ki,ki->k", mu, h)
    cb = cp - cp.max() + SHIFT

    # A1s [128, NT, 128]: rows d' (dup 0-63/64-127), col p = 8k+ds
    A1 = np.zeros((64, NT, 128), dtype=np.float32)
    A2s = np.zeros((128, NT, 64), dtype=np.float32)
    for k in range(K):
        for ds in range(DS):
            p = 8 * k + ds
            for tt in range(NT):
                A1[:, tt, p] = Linv[k, 8 * tt + ds, :]
                A2s[p, tt, :] = sqrt1m * Linv[k, 8 * tt + ds, :]
    A1s = np.concatenate([A1, A1], axis=0)

    onesblk = np.zeros((128, K), dtype=np.float32)
    for k in range(K):
        onesblk[8 * k : 8 * k + 8, k] = -0.5  # fold -0.5 into the reduce
    H2c = h.T.astype(np.float32)               # [64, K]
    ERep = np.zeros((K, 128), dtype=np.float32)
    for k in range(K):
        ERep[k, 8 * k : 8 * k + 8] = 1.0
    OnesRep = np.ones((K, 128), dtype=np.float32)
    negHsRep = np.zeros((128, 64), dtype=np.float32)
    for k in range(K):
        negHsRep[8 * k : 8 * k + 8, :] = (-sqrt1m / DS) * h[k, :]

    blob = np.zeros((128, CBLOB_W), dtype=np.float32)
    blob[:, _A1 : _A1 + 1024] = A1s.reshape(128, 1024)
    blob[:, _ONB : _ONB + K] = onesblk
    blob[0:64, _H2C : _H2C + K] = H2c
    blob[0:K, _EREP : _EREP + 128] = ERep
    blob[0:K, _ONR : _ONR + 128] = OnesRep
    blob[:, _NHR : _NHR + 64] = negHsRep
    blob[0:K, _CB] = cb.astype(np.float32)
    blob[0:32, _ID : _ID + 32] = np.eye(32, dtype=np.float32)
    cbf = A2s.reshape(128, NT * 64).astype(np.float32)
    import ml_dtypes
    cbf = cbf.astype(ml_dtypes.bfloat16)
    return dict(cblob=blob, cbf=cbf)


def _build_bass(nchunk=NCHUNK):
    nc = bass.Bass()
    xT_in = nc.declare_dram_parameter("xT", [D, BP], F32R, isOutput=False)
    outT = nc.declare_dram_parameter("outT", [D, BP], F32, isOutput=True)
    c_blob = nc.declare_dram_parameter("cblob", [128, CBLOB_W], F32R,
                                       isOutput=False)
    c_bf = nc.declare_dram_parameter("cbf", [128, NT * 64], BF16,
                                     isOutput=False)

    xv = xT_in.rearrange("d (n b) -> n d b", b=NB)
    ovT = outT.rearrange("d (n b) -> n d b", b=NB)

    r = lambda ap: ap.bitcast(F32R)  # noqa: E731

    with tile.TileContext(nc) as tc:
        with (
            tc.tile_pool(name="consts", bufs=1) as consts,
            tc.tile_pool(name="xin", bufs=3) as xpool,
            tc.tile_pool(name="sq", bufs=2) as sq_pool,
            tc.tile_pool(name="small", bufs=2) as small_pool,
            tc.tile_pool(name="embp", bufs=2) as emb_pool,
            tc.tile_pool(name="wbuf", bufs=2) as w_pool,
            tc.tile_pool(name="obuf", bufs=3) as o_pool,
            tc.tile_pool(name="zpsum", bufs=5, space="PSUM") as zpsum,
            tc.tile_pool(name="pmpsum", bufs=1, space="PSUM") as pmpsum,
            tc.tile_pool(name="empsum", bufs=1, space="PSUM") as empsum,
            tc.tile_pool(name="popsum", bufs=1, space="PSUM") as popsum,
        ):
            cblob = consts.tile([128, CBLOB_W], F32R)
            nc.sync.dma_start(out=cblob, in_=c_blob[...])
            cbf = consts.tile([128, NT, 64], BF16)
            nc.sync.dma_start(out=cbf, in_=c_bf[...].rearrange(
                "p (t c) -> p t c", t=NT))
            A1s = cblob[:, _A1 : _A1 + 1024].rearrange(
                "p (t c) -> p t c", t=NT)
            onesblk = cblob[:, _ONB : _ONB + K]
            H2c = cblob[0:64, _H2C : _H2C + K]
            ERep = cblob[0:K, _EREP : _EREP + 128]
            OnesRep = cblob[0:K, _ONR : _ONR + 128]
            negHsRep = cblob[:, _NHR : _NHR + 64]
            cb = cblob[0:K, _CB : _CB + 1].bitcast(F32)
            ident = cblob[0:32, _ID : _ID + 32]

            # PE warmup read of cblob so later matmuls don't each need a
            # DMA wait (walrus allows only one sync-wait per instruction);
            # shares the z pool slot so no extra PSUM bank is needed.
            pwarm = zpsum.tile([128, NB], F32, tag="z")
            nc.tensor.matmul(
                pwarm[0:32, 0:32].bitcast(F32R), ident, ident,
                is_transpose=True,
            )
            # touch cbf too (one tiny bf16 matmul into the warm slot)
            nc.tensor.matmul(
                pwarm[0:64, 64:128], cbf[:, 0, :], cbf[:, 0, :],
                start=True, stop=True,
            )

            # --- software-pipelined chunk loop -------------------------
            # PE emission order per step: s-mm(i), ERep(i), Z2(i),
            # A(i+1) [=Z1+maha+exp1], mm2(i).  Next-chunk phase-1 matmuls
            # fill the PE while chunk i's softmax/W run on ACT/DVE, so the
            # PE never idles long enough for HAM to re-throttle.
            xts_t = {}
            st = {}

            def load_x(j):
                if j >= nchunk:
                    return
                xb = xpool.tile([128, NB], F32R, tag="x")
                nc.sync.dma_start(out=xb[0:64, :], in_=xv[j])
                if ROWALT:
                    nc.sync.dma_start(out=xb[64:128, :], in_=xv[j])
                xts_t[j] = xb

            def stageA(j):
                """Z1 waves + squares + maha + h.x + exp1 for chunk j."""
                xbuf = xts_t[j]
                sq = sq_pool.tile([128, NT, NB], F32R, tag="sq")
                for t in range(NT):
                    zw = zpsum.tile([128, NB], F32, tag="z")
                    rlo = 64 * (t % 2) if ROWALT else 0
                    nc.tensor.matmul(
                        zw,
                        A1s[rlo : rlo + 64, t, :],
                        xbuf[rlo : rlo + 64, :],
                        start=True, stop=True,
                    )
                    nc.scalar.square(sq[:, t, :], zw)
                pm = pmpsum.tile([16, NB], F32, tag="pm")
                for t in range(NT):
                    nc.tensor.matmul(
                        pm, onesblk, sq[:, t, :],
                        start=(t == 0), stop=False,
                    )
                nc.tensor.matmul(pm, H2c, xbuf[0:64, :],
                                 start=False, stop=True)
                eT = small_pool.tile([16, NB], F32R, tag="eT")
                nc.scalar.activation(
                    eT, pm, mybir.ActivationFunctionType.Exp,
                    bias=cb, scale=1.0,
                )
                st[j] = dict(pm=pm, eT=eT)

            def stageS1(j):
                """s = sum_k e (replicated), ln s, lnorm = lT + c' - ln s."""
                srep = empsum.tile([128, NB], F32, tag="em")
                nc.tensor.matmul(srep, OnesRep, st[j]["eT"],
                                 start=True, stop=True)
                logS = small_pool.tile([16, NB], F32, tag="logS")
                nc.scalar.activation(
                    logS, srep[0:16, :], mybir.ActivationFunctionType.Ln,
                )
                lnorm = small_pool.tile([16, NB], F32R, tag="lnorm")
                nc.vector.scalar_tensor_tensor(
                    lnorm, st[j]["pm"], cb, logS,
                    op0=mybir.AluOpType.add, op1=mybir.AluOpType.subtract,
                )
                st[j]["lnorm"] = lnorm

            def stageS2(j):
                """replicate lnorm to 128 partitions, exp -> r replicated."""
                em = empsum.tile([128, NB], F32, tag="em")
                nc.tensor.matmul(em, ERep, st[j]["lnorm"],
                                 start=True, stop=True)
                emb = emb_pool.tile([128, NB], F32R, tag="emb")
                nc.scalar.activation(
                    emb, em, mybir.ActivationFunctionType.Exp,
                )
                st[j]["emb"] = emb

            def stageZ2W(j):
                """recompute Z per wave, W = Z*r (bf16 out)."""
                xbuf = xts_t[j]
                emb = st[j]["emb"]
                wb = w_pool.tile([128, NT, NB], BF16, tag="wb")
                for t in range(NT):
                    zw = zpsum.tile([128, NB], F32, tag="z")
                    rlo = 64 * (t % 2) if ROWALT else 0
                    nc.tensor.matmul(
                        zw,
                        A1s[rlo : rlo + 64, t, :],
                        xbuf[rlo : rlo + 64, :],
                        start=True, stop=True,
                    )
                    nc.vector.tensor_tensor(
                        wb[:, t, :], zw, emb, mybir.AluOpType.mult,
                    )
                st[j]["wb"] = wb

            def stageOut(j):
                """negHsRep on r-replicated + mm2 (bf16), evac, store."""
                emb = st[j]["emb"]
                wb = st[j]["wb"]
                po = popsum.tile([128, NB], F32, tag="po")
                if COLTILE:
                    nc.tensor.matmul(po[0:64, :], negHsRep, emb,
                                     start=True, stop=False)
                    for t in range(NT):
                        half = 64 * (t % 2)
                        nc.tensor.matmul(
                            po[half : half + 64, :], cbf[:, t, :],
                            wb[:, t, :],
                            start=(t == 1), stop=(t >= NT - 2),
                        )
                    osb = o_pool.tile([64, NB], F32, tag="osb")
                    nc.vector.tensor_tensor(
                        osb, po[0:64, :], po[64:128, :], mybir.AluOpType.add,
                    )
                else:
                    nc.tensor.matmul(po[0:64, :], negHsRep, emb,
                                     start=True, stop=False)
                    for t in range(NT):
                        nc.tensor.matmul(
                            po[0:64, :], cbf[:, t, :], wb[:, t, :],
                            start=False, stop=(t == NT - 1),
                        )
                    osb = o_pool.tile([64, NB], F32, tag="osb")
                    nc.vector.tensor_copy(osb, po[0:64, :])
                nc.sync.dma_start(out=ovT[j], in_=osb)
                del st[j]

            load_x(0)
            load_x(1)
            stageA(0)
            for i in range(nchunk):
                stageS1(i)
                stageS2(i)
                stageZ2W(i)
                load_x(i + 2)
                if i + 1 < nchunk:
                    stageA(i + 1)
                stageOut(i)

    return nc


def _legalize_waits(bir_bytes: bytes) -> bytes:
    """Walrus codegen allows at most ONE sync-wait per instruction. Tile's
    scheduler can emit several (one per upstream proc). Split the extras
    into standalone EventSemaphore instructions on the same engine, placed
    immediately before -- the engine sequencer executes them in order, so
    semantics are preserved."""
    import json as _json

    bir = _json.loads(bir_bytes)
    n_new = 0
    for fn in bir["functions"]:
        for blk in fn["blocks"]:
            insts = blk.get("instructions", [])
            out = []
            for inst in insts:
                si = inst.get("sync_info")
                waits = (si or {}).get("on_wait") or []
                if len(waits) > 1:
                    for w in waits[:-1]:
                        n_new += 1
                        out.append({
                            "debug": inst.get("debug", 0),
                            "engine": inst["engine"],
                            "ins": [],
                            "name": f"I-waitsplit-{n_new}",
                            "opcode": "EventSemaphore",
                            "outs": [],
                            "sync_info": {"on_update": [], "on_wait": [w]},
                        })
                    si["on_wait"] = [waits[-1]]
                out.append(inst)
            blk["instructions"] = out
    return _json.dumps(bir).encode()


def _install_wait_legalizer():
    from concourse import bass2jax as _b2j
    from concourse import bass_utils as _bu

    if getattr(_b2j, "_wait_legalizer_installed", False):
        return
    _orig = _bu.compile_bir_kernel

    def _patched(bir_bytes, compile_dir_path, neff_name="file.neff", **kw):
        return _orig(_legalize_waits(bir_bytes), compile_dir_path,
                     neff_name=neff_name, **kw)

    _b2j.compile_bir_kernel = _patched
    _b2j._wait_legalizer_installed = True


_NC_CACHE = None


def _prep_in_maps(x, means, weights, covs, alphas_cumprod, t):
    x = np.ascontiguousarray(np.asarray(x, dtype=np.float32))
    consts = _host_precompute(
        np.asarray(means, dtype=np.float32),
        np.asarray(weights, dtype=np.float32),
        np.asarray(covs, dtype=np.float32),
        np.asarray(alphas_cumprod, dtype=np.float32),
        int(np.asarray(t)),
    )
    in_maps = []
    for c in range(NCORES):
        xT = np.ascontiguousarray(x[c * BP : (c + 1) * BP].T)
        m = {"xT": xT}
        m.update(consts)
        in_maps.append(m)
    return in_maps


def kernel(x, means, weights, covs, alphas_cumprod, t):
    global _NC_CACHE
    if _NC_CACHE is None:
        _NC_CACHE = _build_bass()
    in_maps = _prep_in_maps(x, means, weights, covs, alphas_cumprod, t)
    _install_wait_legalizer()
    res = run_bass_kernel_spmd(_NC_CACHE, in_maps, list(range(NCORES)))
    outs = [res.results[c]["outT"].T for c in range(NCORES)]
    return np.ascontiguousarray(np.concatenate(outs, axis=0), dtype=np.float32)


def run_traced(inputs, trace=True, tmpdir=None):
    """Run once with NTFF tracing; returns BassKernelResults (exec_time_ns)."""
    global _NC_CACHE
    if _NC_CACHE is None:
        _NC_CACHE = _build_bass()
    in_maps = _prep_in_maps(
        inputs["x"], inputs["means"], inputs["weights"], inputs["covs"],
        inputs["alphas_cumprod"], inputs["t"],
    )
    _install_wait_legalizer()
    return run_bass_kernel_spmd(
        _NC_CACHE, in_maps, list(range(NCORES)), trace=trace, tmpdir=tmpdir
    )


# revision 18
# speedup vs baseline: 1.2700x; 1.2700x over previous
"""Trainium2 Bass kernel for EpsilonNetGM score function (8-core data parallel).

Closed form of the score (no autodiff):
  acp = alphas_cumprod[t]; mu_k = sqrt(acp)*means_k
  Sigma_k = (1-acp) I + acp covs_k ; L = chol(Sigma); Linv = L^-1; P = Linv^T Linv
  z_k(x) = Linv_k x
  l_k(x) = -0.5|z_k|^2 + (P_k mu_k).x + c'_k        (c' folds logdet, weights, mu)
  r = softmax_k(l)     (computed as exp(l - logsumexp l), no per-column max --
                        a global shift keeps exp in fp32 range)
  out = sqrt(1-acp) * [ sum_k Linv_k^T (r_k z_k) - sum_k r_k (P_k mu_k) ]

v2 layout: x arrives HOST-TRANSPOSED as xT [64, BP] so no on-device transpose.
Partition p = 8k+ds (ds in [0,8)), d = 8t+ds over NT=8 subtiles; 512-col chunks.
Per chunk:
  PE : Z waves (fp32r), maha ones-block reduce + h.x into lT [16,512] psum,
       s-sum matmul, ERep replicate of lnorm, Z recompute, negHsRep + mm2 (bf16)
  ACT: squares (PSUM->SBUF), exp(lT + c'), ln(s), exp(lnorm rep) -> r replicated
  DVE: lnorm = lT - ln(s), W = Z*r (bf16 out), final psum evac
"""

import math
import sys

import numpy as np

sys.path.insert(0, "/opt/trn_rl_repo")

import concourse.bass as bass  # noqa: E402
import concourse.tile as tile  # noqa: E402
from concourse import mybir  # noqa: E402
from concourse.bass_utils import run_bass_kernel_spmd  # noqa: E402

B, K, D, T = 65536, 16, 64, 1000
NCORES = 8
BP = B // NCORES          # rows per core = 8192
NB = 512                  # batch chunk (free dim)
NCHUNK = BP // NB         # 16
DS = 8                    # d-subtile width; partition p = 8*k + ds
NT = D // DS              # 8 subtiles
SHIFT = 40.0              # global exp shift; keeps exp(l) in fp32 range

COLTILE = True            # col-tiled mm2 with both-psum final add

F32 = mybir.dt.float32
F32R = mybir.dt.float32r
BF16 = mybir.dt.bfloat16

# f32 cblob column layout
_ONB, _EREP, _NHR, _CB, _ID = (0, 16, 144, 208, 209)
CBLOB_W = _ID + 32        # 241
# bf16 cbf column layout: A2s | A1 hi/lo pairs | H2c hi/lo | OnesRep
_BA2, _BA1, _BH2, _BONR = (0, 512, 2560, 2576)
CBF_W = _BONR + 128       # 2704


def _host_precompute(means, weights, covs, alphas_cumprod, t):
    acp = float(np.asarray(alphas_cumprod)[int(t)])
    s1 = math.sqrt(acp)
    sqrt1m = math.sqrt(1.0 - acp)
    mu = (s1 * means).astype(np.float64)
    covs = covs.astype(np.float64)
    sigma = (1.0 - acp) * np.eye(D) + acp * covs
    chol = np.linalg.cholesky(sigma)
    Linv = np.stack([np.linalg.solve(chol[k], np.eye(D)) for k in range(K)])
    P = np.einsum("kdi,kdj->kij", Linv, Linv)
    h = np.einsum("kij,kj->ki", P, mu)
    logdet = 2.0 * np.log(np.diagonal(chol, axis1=1, axis2=2)).sum(-1)
    w = weights.astype(np.float64)
    logw = np.log(w) - math.log(w.sum())
    c = logw - 0.5 * (D * math.log(2 * math.pi) + logdet)
    cp = c - 0.5 * np.einsum("ki,ki->k", mu, h)
    cb = cp - cp.max() + SHIFT

    import ml_dtypes
    bfd = ml_dtypes.bfloat16

    def tobf(a):
        return np.asarray(a, dtype=np.float32).astype(bfd).astype(np.float32)

    # A1 [64, NT, 128]: rows d', col p = 8k+ds; split hi/lo bf16 for
    # fp32-quality Z via two row-tiled bf16 matmuls accumulating in PSUM
    A1 = np.zeros((64, NT, 128), dtype=np.float32)
    A2s = np.zeros((128, NT, 64), dtype=np.float32)
    for k in range(K):
        for ds in range(DS):
            p = 8 * k + ds
            for tt in range(NT):
                A1[:, tt, p] = Linv[k, 8 * tt + ds, :]
                A2s[p, tt, :] = sqrt1m * Linv[k, 8 * tt + ds, :]
    A1h = tobf(A1)
    A1l = A1 - A1h

    onesblk = np.zeros((128, K), dtype=np.float32)
    for k in range(K):
        onesblk[8 * k : 8 * k + 8, k] = -0.5  # fold -0.5 into the reduce
    H2cf = h.T.astype(np.float32)              # [64, K]
    H2ch = tobf(H2cf)
    H2cl = H2cf - H2ch
    ERep = np.zeros((K, 128), dtype=np.float32)
    for k in range(K):
        ERep[k, 8 * k : 8 * k + 8] = 1.0
    OnesRep = np.ones((K, 128), dtype=np.float32)
    negHsRep = np.zeros((128, 64), dtype=np.float32)
    for k in range(K):
        negHsRep[8 * k : 8 * k + 8, :] = (-sqrt1m / DS) * h[k, :]

    blob = np.zeros((128, CBLOB_W), dtype=np.float32)
    blob[:, _ONB : _ONB + K] = onesblk
    blob[0:K, _EREP : _EREP + 128] = ERep
    blob[:, _NHR : _NHR + 64] = negHsRep
    blob[0:K, _CB] = cb.astype(np.float32)
    blob[0:32, _ID : _ID + 32] = np.eye(32, dtype=np.float32)

    # A1 hi/lo pairs: subtile t lives on row half 64*(t%2); within a
    # half, j=0 holds the bf16 hi part, j=1 the bf16 lo residual.  The
    # hi+lo matmuls accumulate sequentially in ONE row group (safe
    # split-K); consecutive subtiles alternate halves so their pairs run
    # concurrently on disjoint PE row groups.
    A1p = np.zeros((128, NT, 2, 128), dtype=np.float32)
    for tt in range(NT):
        rlo = 64 * (tt % 2)
        A1p[rlo : rlo + 64, tt, 0, :] = A1h[:, tt, :]
        A1p[rlo : rlo + 64, tt, 1, :] = A1l[:, tt, :]
    cbf = np.zeros((128, CBF_W), dtype=np.float32)
    cbf[:, _BA2 : _BA2 + 512] = A2s.reshape(128, 512)
    cbf[:, _BA1 : _BA1 + 2048] = A1p.reshape(128, 2048)
    cbf[0:64, _BH2 : _BH2 + K] = H2ch
    cbf[0:64, _BH2 + K : _BH2 + 2 * K] = H2cl
    cbf[0:K, _BONR : _BONR + 128] = OnesRep
    cbf = cbf.astype(bfd)
    return dict(cblob=blob, cbf=cbf)


def _build_bass(nchunk=NCHUNK):
    nc = bass.Bass()
    xT_in = nc.declare_dram_parameter("xT", [D, BP], BF16, isOutput=False)
    outT = nc.declare_dram_parameter("outT", [D, BP], F32, isOutput=True)
    c_blob = nc.declare_dram_parameter("cblob", [128, CBLOB_W], F32R,
                                       isOutput=False)
    c_bf = nc.declare_dram_parameter("cbf", [128, CBF_W], BF16,
                                     isOutput=False)

    xv = xT_in.rearrange("d (n b) -> n d b", b=NB)
    ovT = outT.rearrange("d (n b) -> n d b", b=NB)

    r = lambda ap: ap.bitcast(F32R)  # noqa: E731

    with tile.TileContext(nc) as tc:
        with (
            tc.tile_pool(name="consts", bufs=1) as consts,
            tc.tile_pool(name="xin", bufs=3) as xpool,
            tc.tile_pool(name="sq", bufs=2) as sq_pool,
            tc.tile_pool(name="small", bufs=2) as small_pool,
            tc.tile_pool(name="embp", bufs=2) as emb_pool,
            tc.tile_pool(name="wbuf", bufs=2) as w_pool,
            tc.tile_pool(name="obuf", bufs=3) as o_pool,
            tc.tile_pool(name="zpsum", bufs=5, space="PSUM") as zpsum,
            tc.tile_pool(name="pmpsum", bufs=1, space="PSUM") as pmpsum,
            tc.tile_pool(name="empsum", bufs=1, space="PSUM") as empsum,
            tc.tile_pool(name="popsum", bufs=1, space="PSUM") as popsum,
        ):
            cblob = consts.tile([128, CBLOB_W], F32R)
            nc.sync.dma_start(out=cblob, in_=c_blob[...])
            cbf = consts.tile([128, CBF_W], BF16)
            nc.sync.dma_start(out=cbf, in_=c_bf[...])
            A2bf = cbf[:, _BA2 : _BA2 + 512].rearrange(
                "p (t c) -> p t c", t=NT)
            A1bf = cbf[:, _BA1 : _BA1 + 2048].rearrange(
                "p (t j c) -> p t j c", t=NT, j=2)
            H2bf = cbf[0:64, _BH2 : _BH2 + 2 * K].rearrange(
                "p (j c) -> p j c", j=2)
            OnesRepBf = cbf[0:K, _BONR : _BONR + 128]
            onesblk = cblob[:, _ONB : _ONB + K]
            ERep = cblob[0:K, _EREP : _EREP + 128]
            negHsRep = cblob[:, _NHR : _NHR + 64]
            cb = cblob[0:K, _CB : _CB + 1].bitcast(F32)
            ident = cblob[0:32, _ID : _ID + 32]

            # PE warmup read of cblob so later matmuls don't each need a
            # DMA wait (walrus allows only one sync-wait per instruction);
            # shares the z pool slot so no extra PSUM bank is needed.
            pwarm = zpsum.tile([128, NB], F32, tag="z")
            nc.tensor.matmul(
                pwarm[0:32, 0:32].bitcast(F32R), ident, ident,
                is_transpose=True,
            )
            # touch cbf too (one tiny bf16 matmul into the warm slot)
            nc.tensor.matmul(
                pwarm[0:64, 64:128], A2bf[:, 0, :], A2bf[:, 0, :],
                start=True, stop=True,
            )

            # --- software-pipelined chunk loop -------------------------
            # PE emission order per step: s-mm(i), ERep(i), Z2(i),
            # A(i+1) [=Z1+maha+exp1], mm2(i).  Next-chunk phase-1 matmuls
            # fill the PE while chunk i's softmax/W run on ACT/DVE, so the
            # PE never idles long enough for HAM to re-throttle.
            xts_t = {}
            st = {}

            def load_x(j):
                if j >= nchunk:
                    return
                xb = xpool.tile([128, NB], BF16, tag="x")
                nc.sync.dma_start(out=xb[0:64, :], in_=xv[j])
                nc.sync.dma_start(out=xb[64:128, :], in_=xv[j])
                xts_t[j] = xb

            def stageA(j):
                """Z1 waves + squares + maha + h.x + exp1 for chunk j."""
                xbuf = xts_t[j]
                sq = sq_pool.tile([128, NT, NB], F32R, tag="sq")
                for t in range(NT):
                    zw = zpsum.tile([128, NB], F32, tag="z")
                    rlo = 64 * (t % 2)
                    nc.tensor.matmul(
                        zw, A1bf[rlo : rlo + 64, t, 0, :],
                        xbuf[rlo : rlo + 64, :],
                        start=True, stop=False,
                    )
                    nc.tensor.matmul(
                        zw, A1bf[rlo : rlo + 64, t, 1, :],
                        xbuf[rlo : rlo + 64, :],
                        start=False, stop=True,
                    )
                    nc.scalar.square(sq[:, t, :], zw)
                pm = pmpsum.tile([16, NB], F32, tag="pm")
                for t in range(NT):
                    nc.tensor.matmul(
                        pm, onesblk, sq[:, t, :],
                        start=(t == 0), stop=False,
                    )
                nc.tensor.matmul(pm, H2bf[:, 0, :], xbuf[0:64, :],
                                 start=False, stop=False)
                nc.tensor.matmul(pm, H2bf[:, 1, :], xbuf[0:64, :],
                                 start=False, stop=True)
                eT = small_pool.tile([16, NB], BF16, tag="eT")
                nc.scalar.activation(
                    eT, pm, mybir.ActivationFunctionType.Exp,
                    bias=cb, scale=1.0,
                )
                st[j] = dict(pm=pm, eT=eT)

            def stageS1(j):
                """s = sum_k e (replicated), ln s, lnorm = lT + c' - ln s."""
                srep = empsum.tile([128, NB], F32, tag="em")
                nc.tensor.matmul(srep, OnesRepBf, st[j]["eT"],
                                 start=True, stop=True)
                logS = small_pool.tile([16, NB], F32, tag="logS")
                nc.scalar.activation(
                    logS, srep[0:16, :], mybir.ActivationFunctionType.Ln,
                )
                lnorm = small_pool.tile([16, NB], F32R, tag="lnorm")
                nc.vector.scalar_tensor_tensor(
                    lnorm, st[j]["pm"], cb, logS,
                    op0=mybir.AluOpType.add, op1=mybir.AluOpType.subtract,
                )
                st[j]["lnorm"] = lnorm

            def stageS2(j):
                """replicate lnorm to 128 partitions, exp -> r replicated."""
                em = empsum.tile([128, NB], F32, tag="em")
                nc.tensor.matmul(em, ERep, st[j]["lnorm"],
                                 start=True, stop=True)
                emb = emb_pool.tile([128, NB], F32R, tag="emb")
                nc.scalar.activation(
                    emb, em, mybir.ActivationFunctionType.Exp,
                )
                st[j]["emb"] = emb

            def stageZ2W(j):
                """recompute Z per wave, W = Z*r (bf16 out)."""
                xbuf = xts_t[j]
                emb = st[j]["emb"]
                wb = w_pool.tile([128, NT, NB], BF16, tag="wb")
                for t in range(NT):
                    zw = zpsum.tile([128, NB], F32, tag="z")
                    rlo = 64 * (t % 2)
                    nc.tensor.matmul(
                        zw, A1bf[rlo : rlo + 64, t, 0, :],
                        xbuf[rlo : rlo + 64, :],
                        start=True, stop=False,
                    )
                    nc.tensor.matmul(
                        zw, A1bf[rlo : rlo + 64, t, 1, :],
                        xbuf[rlo : rlo + 64, :],
                        start=False, stop=True,
                    )
                    nc.vector.tensor_tensor(
                        wb[:, t, :], zw, emb, mybir.AluOpType.mult,
                    )
                st[j]["wb"] = wb

            def stageOut(j):
                """negHsRep on r-replicated + mm2 (bf16), evac, store."""
                emb = st[j]["emb"]
                wb = st[j]["wb"]
                po = popsum.tile([128, NB], F32, tag="po")
                if COLTILE:
                    nc.tensor.matmul(po[0:64, :], negHsRep, emb,
                                     start=True, stop=False)
                    for t in range(NT):
                        half = 64 * (t % 2)
                        nc.tensor.matmul(
                            po[half : half + 64, :], A2bf[:, t, :],
                            wb[:, t, :],
                            start=(t == 1), stop=(t >= NT - 2),
                        )
                    ohalf = o_pool.tile([64, NB], F32, tag="ohalf")
                    nc.vector.tensor_copy(ohalf, po[64:128, :])
                    osb = o_pool.tile([64, NB], F32, tag="osb")
                    nc.vector.tensor_tensor(
                        osb, po[0:64, :], ohalf, mybir.AluOpType.add,
                    )
                else:
                    nc.tensor.matmul(po[0:64, :], negHsRep, emb,
                                     start=True, stop=False)
                    for t in range(NT):
                        nc.tensor.matmul(
                            po[0:64, :], A2bf[:, t, :], wb[:, t, :],
                            start=False, stop=(t == NT - 1),
                        )
                    osb = o_pool.tile([64, NB], F32, tag="osb")
                    nc.vector.tensor_copy(osb, po[0:64, :])
                nc.sync.dma_start(out=ovT[j], in_=osb)
                del st[j]

            load_x(0)
            load_x(1)
            stageA(0)
            for i in range(nchunk):
                stageS1(i)
                stageS2(i)
                stageZ2W(i)
                load_x(i + 2)
                if i + 1 < nchunk:
                    stageA(i + 1)
                stageOut(i)

    return nc


def _legalize_waits(bir_bytes: bytes) -> bytes:
    """Walrus codegen allows at most ONE sync-wait per instruction. Tile's
    scheduler can emit several (one per upstream proc). Split the extras
    into standalone EventSemaphore instructions on the same engine, placed
    immediately before -- the engine sequencer executes them in order, so
    semantics are preserved."""
    import json as _json

    bir = _json.loads(bir_bytes)
    n_new = 0
    for fn in bir["functions"]:
        for blk in fn["blocks"]:
            insts = blk.get("instructions", [])
            out = []
            for inst in insts:
                si = inst.get("sync_info")
                waits = (si or {}).get("on_wait") or []
                if len(waits) > 1:
                    for w in waits[:-1]:
                        n_new += 1
                        out.append({
                            "debug": inst.get("debug", 0),
                            "engine": inst["engine"],
                            "ins": [],
                            "name": f"I-waitsplit-{n_new}",
                            "opcode": "EventSemaphore",
                            "outs": [],
                            "sync_info": {"on_update": [], "on_wait": [w]},
                        })
                    si["on_wait"] = [waits[-1]]
                out.append(inst)
            blk["instructions"] = out
    return _json.dumps(bir).encode()


def _install_wait_legalizer():
    from concourse import bass2jax as _b2j
    from concourse import bass_utils as _bu

    if getattr(_b2j, "_wait_legalizer_installed", False):
        return
    _orig = _bu.compile_bir_kernel

    def _patched(bir_bytes, compile_dir_path, neff_name="file.neff", **kw):
        return _orig(_legalize_waits(bir_bytes), compile_dir_path,
                     neff_name=neff_name, **kw)

    _b2j.compile_bir_kernel = _patched
    _b2j._wait_legalizer_installed = True


_NC_CACHE = None


def _prep_in_maps(x, means, weights, covs, alphas_cumprod, t):
    x = np.ascontiguousarray(np.asarray(x, dtype=np.float32))
    consts = _host_precompute(
        np.asarray(means, dtype=np.float32),
        np.asarray(weights, dtype=np.float32),
        np.asarray(covs, dtype=np.float32),
        np.asarray(alphas_cumprod, dtype=np.float32),
        int(np.asarray(t)),
    )
    import ml_dtypes
    in_maps = []
    for c in range(NCORES):
        xT = np.ascontiguousarray(
            x[c * BP : (c + 1) * BP].T).astype(ml_dtypes.bfloat16)
        m = {"xT": xT}
        m.update(consts)
        in_maps.append(m)
    return in_maps


def kernel(x, means, weights, covs, alphas_cumprod, t):
    global _NC_CACHE
    if _NC_CACHE is None:
        _NC_CACHE = _build_bass()
    in_maps = _prep_in_maps(x, means, weights, covs, alphas_cumprod, t)
    _install_wait_legalizer()
    res = run_bass_kernel_spmd(_NC_CACHE, in_maps, list(range(NCORES)))
    outs = [res.results[c]["outT"].T for c in range(NCORES)]
    return np.ascontiguousarray(np.concatenate(outs, axis=0), dtype=np.float32)


def run_traced(inputs, trace=True, tmpdir=None):
    """Run once with NTFF tracing; returns BassKernelResults (exec_time_ns)."""
    global _NC_CACHE
    if _NC_CACHE is None:
        _NC_CACHE = _build_bass()
    in_maps = _prep_in_maps(
        inputs["x"], inputs["means"], inputs["weights"], inputs["covs"],
        inputs["alphas_cumprod"], inputs["t"],
    )
    _install_wait_legalizer()
    return run_bass_kernel_spmd(
        _NC_CACHE, in_maps, list(range(NCORES)), trace=trace, tmpdir=tmpdir
    )
